# revision 3
# baseline (speedup 1.0000x reference)
"""BitLinear kernel for Trainium2, 8 NeuronCores, column-parallel.

y[t, o] = sum_i x[t, i] * sign(W[o, i]) * scale[o]
  x: [8192, 4096] f32 (replicated), W: [16384, 4096] f32, scale: [16384] f32
  Each core owns OUT_F/8 = 2048 output features (column parallel).

v3: the PE does ONLY the 4.19M cycles of fp16 matmul (roofline 1.75 ms
@2.4GHz); every transpose runs on the DMA XBAR (InstDmaTransposeAnt,
16x128 tiles) and W sign-prep runs on DVE:
  - W prep:   f32 --SWDGE casting DMA--> w16 f16 SBUF; sign(w)*scale via
              one DVE tensor_scalar: (w16 & 0x8000) ^ bits(f16(scale[o]))
              -> +-scale exactly; DMA-transpose (sync ring) -->
              B [128, 32, 2048] f16 resident.
  - x path:   x f32 --SWDGE cast--> DRAM f16 scratch tiles (pipeline can
              run far ahead, no SBUF pressure); one DMA-transpose per
              token tile (scalar ring) DRAM -> xT [128, 32, 128] f16.
  - matmul:   per 128-token tile and 512-out band, 32 fp16 matmuls
              accumulate K into PSUM [128, 512] f32 -> DVE copy -> DMA out.
Band-major warm phase lets W-band b+1 stream in on DMA/DVE while the PE
chews band b; no PE cycles are ever spent on prep.  fp16 keeps
sign*scale exact; only x quantizes (~2e-4 rel err); PSUM accums in f32.
"""

import os
import sys

for _p in ("/opt/trn_rl_repo",):
    if _p not in sys.path and os.path.isdir(_p):
        sys.path.append(_p)

import numpy as np
import concourse.bacc as bacc
import concourse.mybir as mybir
from concourse.tile import TileContext
from concourse.bass_utils import run_bass_kernel_spmd

TOKENS, IN_F, OUT_F, NCORES = 8192, 4096, 16384, 8
O_SH = OUT_F // NCORES  # 2048 out features per core
P = 128
KT = IN_F // P          # 32 k-subtiles
MT = TOKENS // P        # 64 token tiles
OT = O_SH // P          # 16 o-tiles per core
W_KC = 2048             # W prep k-chunk (per o-tile)
NKC = IN_F // W_KC      # 2 k-chunks per o-tile
NBAND = 4               # 4 output bands of 512
WARM = 3                # band-major warm token tiles
XAHEAD = 3              # x cast lookahead (token tiles)

f32, f16, u16 = mybir.dt.float32, mybir.dt.float16, mybir.dt.uint16
AF = mybir.ActivationFunctionType

_CACHE = {}
last_result = None


def build():
    nc = bacc.Bacc("TRN2", target_bir_lowering=False, debug=False)
    x = nc.dram_tensor("x", [TOKENS, IN_F], f32, kind="ExternalInput").ap()
    w = nc.dram_tensor("weight", [O_SH, IN_F], f32, kind="ExternalInput").ap()
    scale = nc.dram_tensor("scale", [O_SH], f32, kind="ExternalInput").ap()
    y = nc.dram_tensor("y", [TOKENS, O_SH], f32, kind="ExternalOutput").ap()

    with TileContext(nc) as tc:
        with (
            tc.tile_pool(name="const", bufs=1) as cpool,
            tc.tile_pool(name="bres", bufs=1) as bpool,
            tc.tile_pool(name="w16", bufs=2) as w16pool,
            tc.tile_pool(name="xdram", bufs=2 * XAHEAD + 2, space="DRAM") as xdpool,
            tc.tile_pool(name="xtp", bufs=6) as xtpool,
            tc.tile_pool(name="ystage", bufs=2) as ypool,
            tc.tile_pool(name="mmps", bufs=6, space="PSUM") as mmps,
        ):
            scale_sb = cpool.tile([P, OT], f32, tag="scale")
            nc.sync.dma_start(scale_sb[:], scale.rearrange("(o p) -> p o", p=P))
            scale16 = cpool.tile([P, OT], f16, tag="scale16")
            nc.vector.tensor_copy(scale16[:], scale_sb[:])

            B = bpool.tile([P, KT, O_SH], f16, tag="B")

            def prep_chunk(ot, kc):
                """Produce B[:, kc*16:(kc+1)*16, ot*128:(ot+1)*128]."""
                w16 = w16pool.tile([P, W_KC], f16, tag="w16")
                # f32 -> f16 during the DMA itself (SWDGE cast)
                nc.gpsimd.dma_start(
                    w16[:], w[ot * P : (ot + 1) * P, kc * W_KC : (kc + 1) * W_KC]
                )
                # sign(w)*scale = (w16 & 0x8000) ^ bits(f16(scale[o]))
                nc.vector.tensor_scalar(
                    w16[:].bitcast(u16),
                    w16[:].bitcast(u16),
                    0x8000,
                    scale16[:, ot : ot + 1].bitcast(u16),
                    mybir.AluOpType.bitwise_and,
                    mybir.AluOpType.bitwise_xor,
                )
                ksub0 = kc * (W_KC // P)
                nc.sync.dma_start_transpose(
                    B[:, ksub0 : ksub0 + W_KC // P, ot * P : (ot + 1) * P],
                    w16[:],
                )

            def prep_band(band):
                for kc in range(NKC):
                    for oi in range(4):
                        prep_chunk(band * 4 + oi, kc)

            def xcast(mt):
                xd = xdpool.tile([P, IN_F], f16, tag="xd")
                nc.gpsimd.dma_start(xd[:], x[mt * P : (mt + 1) * P, :])
                return xd

            def xtrans(xd):
                xT = xtpool.tile([P, KT, P], f16, tag="xT")
                nc.scalar.dma_start_transpose(xT[:], xd[:])
                return xT

            def mm_band(mt, band, xT):
                ps = mmps.tile([P, 512], f32, tag="ps")
                n0 = band * 512
                for k in range(KT):
                    nc.tensor.matmul(
                        ps[:],
                        xT[:, k, :],
                        B[:, k, n0 : n0 + 512],
                        start=(k == 0),
                        stop=(k == KT - 1),
                    )
                yq = ypool.tile([P, 512], f32, tag="yq")
                nc.vector.tensor_copy(yq[:], ps[:])
                nc.sync.dma_start(
                    y[mt * P : (mt + 1) * P, n0 : n0 + 512], yq[:]
                )

            # Warm phase: band-major so W band b+1 streams in (DMA/DVE only)
            # while the PE runs band b's matmuls.
            prep_band(0)
            xds = {mt: xcast(mt) for mt in range(WARM + XAHEAD)}
            warm_xT = [xtrans(xds[mt]) for mt in range(WARM)]
            for band in range(NBAND):
                if band + 1 < NBAND:
                    prep_band(band + 1)
                for mt in range(WARM):
                    mm_band(mt, band, warm_xT[mt])

            # steady phase
            for mt in range(WARM, MT):
                if mt + XAHEAD < MT:
                    xds[mt + XAHEAD] = xcast(mt + XAHEAD)
                xT = xtrans(xds.pop(mt))
                for band in range(NBAND):
                    mm_band(mt, band, xT)

    nc.finalize()
    return nc


def _get_nc():
    if "nc" not in _CACHE:
        _CACHE["nc"] = build()
    return _CACHE["nc"]


def kernel(x, weight, scale):
    global last_result
    nc = _get_nc()
    x = np.ascontiguousarray(np.asarray(x, dtype=np.float32))
    weight = np.ascontiguousarray(np.asarray(weight, dtype=np.float32))
    scale = np.ascontiguousarray(np.asarray(scale, dtype=np.float32))
    in_maps = [
        {
            "x": x,
            "weight": np.ascontiguousarray(weight[c * O_SH : (c + 1) * O_SH]),
            "scale": np.ascontiguousarray(scale[c * O_SH : (c + 1) * O_SH]),
        }
        for c in range(NCORES)
    ]
    res = run_bass_kernel_spmd(nc, in_maps, list(range(NCORES)))
    last_result = res
    return np.concatenate([res.results[c]["y"] for c in range(NCORES)], axis=1)


if __name__ == "__main__":
    rng = np.random.default_rng(0)
    xv = rng.standard_normal((TOKENS, IN_F), dtype=np.float32)
    wv = rng.standard_normal((OUT_F, IN_F), dtype=np.float32)
    sv = np.ones(OUT_F, dtype=np.float32)
    yv = kernel(xv, wv, sv)
    print("out shape:", yv.shape, yv.dtype)


# revision 4
# speedup vs baseline: 1.2207x; 1.2207x over previous
"""BitLinear kernel for Trainium2, 8 NeuronCores, column-parallel.

y[t, o] = sum_i x[t, i] * sign(W[o, i]) * scale[o]
  x: [8192, 4096] f32 (replicated), W: [16384, 4096] f32, scale: [16384] f32
  Each core owns OUT_F/8 = 2048 output features (column parallel).

v4: the PE does ONLY the 4.19M cycles of fp16 matmul (roofline 1.75 ms
@2.4GHz); every transpose runs on the DMA XBAR (InstDmaTransposeAnt,
16x128 tiles, ~12ns/tile measured) and W sign-prep runs on DVE:
  - W prep:   f32 --SWDGE casting DMA--> w16 f16 SBUF; sign(w)*scale via
              one DVE tensor_scalar: (w16 & 0x8000) ^ bits(f16(scale[o]))
              -> +-scale exactly (xor only flips f16(scale)'s sign bit);
              DMA-transpose (sync ring) --> B [128, 32, 2048] f16 resident.
  - x path:   x f32 --SWDGE cast--> xc f16 [128, 4096] SBUF; one
              DMA-transpose per token tile (scalar ring) -> xT [128,32,128].
  - matmul:   per 128-token tile and 512-out band, 32 fp16 matmuls
              accumulate K into PSUM [128, 512] f32 -> DVE copy -> DMA out.
Band-major warm phase lets W-band b+1 stream in on DMA/DVE while the PE
chews band b; no PE cycles are ever spent on prep.  fp16 keeps
sign*scale exact; only x quantizes (~2e-4 rel err); PSUM accums in f32.
"""

import os
import sys

for _p in ("/opt/trn_rl_repo",):
    if _p not in sys.path and os.path.isdir(_p):
        sys.path.append(_p)

import numpy as np
import concourse.bacc as bacc
import concourse.mybir as mybir
from concourse.tile import TileContext
from concourse.bass_utils import run_bass_kernel_spmd

TOKENS, IN_F, OUT_F, NCORES = 8192, 4096, 16384, 8
O_SH = OUT_F // NCORES  # 2048 out features per core
P = 128
KT = IN_F // P          # 32 k-subtiles
MT = TOKENS // P        # 64 token tiles
OT = O_SH // P          # 16 o-tiles per core
W_KC = 2048             # W prep k-chunk (per o-tile)
NKC = IN_F // W_KC      # 2 k-chunks per o-tile
NBAND = 4               # 4 output bands of 512
WARM = 3                # band-major warm token tiles

f32, f16, u16 = mybir.dt.float32, mybir.dt.float16, mybir.dt.uint16
AF = mybir.ActivationFunctionType

_CACHE = {}
last_result = None


def build():
    nc = bacc.Bacc("TRN2", target_bir_lowering=False, debug=False)
    x = nc.dram_tensor("x", [TOKENS, IN_F], f32, kind="ExternalInput").ap()
    w = nc.dram_tensor("weight", [O_SH, IN_F], f32, kind="ExternalInput").ap()
    scale = nc.dram_tensor("scale", [O_SH], f32, kind="ExternalInput").ap()
    y = nc.dram_tensor("y", [TOKENS, O_SH], f32, kind="ExternalOutput").ap()

    with TileContext(nc) as tc:
        with (
            tc.tile_pool(name="const", bufs=1) as cpool,
            tc.tile_pool(name="bres", bufs=1) as bpool,
            tc.tile_pool(name="w16", bufs=2) as w16pool,
            tc.tile_pool(name="xstage", bufs=2) as xpool,
            tc.tile_pool(name="xtp", bufs=4) as xtpool,
            tc.tile_pool(name="ystage", bufs=2) as ypool,
            tc.tile_pool(name="mmps", bufs=6, space="PSUM") as mmps,
        ):
            scale_sb = cpool.tile([P, OT], f32, tag="scale")
            nc.sync.dma_start(scale_sb[:], scale.rearrange("(o p) -> p o", p=P))
            scale16 = cpool.tile([P, OT], f16, tag="scale16")
            nc.vector.tensor_copy(scale16[:], scale_sb[:])

            B = bpool.tile([P, KT, O_SH], f16, tag="B")

            def prep_chunk(ot, kc):
                """Produce B[:, kc*16:(kc+1)*16, ot*128:(ot+1)*128]."""
                w16 = w16pool.tile([P, W_KC], f16, tag="w16")
                # f32 -> f16 during the DMA itself (SWDGE cast)
                nc.gpsimd.dma_start(
                    w16[:], w[ot * P : (ot + 1) * P, kc * W_KC : (kc + 1) * W_KC]
                )
                # sign(w)*scale = (w16 & 0x8000) ^ bits(f16(scale[o]))
                nc.vector.tensor_scalar(
                    w16[:].bitcast(u16),
                    w16[:].bitcast(u16),
                    0x8000,
                    scale16[:, ot : ot + 1].bitcast(u16),
                    mybir.AluOpType.bitwise_and,
                    mybir.AluOpType.bitwise_xor,
                )
                ksub0 = kc * (W_KC // P)
                nc.sync.dma_start_transpose(
                    B[:, ksub0 : ksub0 + W_KC // P, ot * P : (ot + 1) * P],
                    w16[:],
                )

            def prep_band(band):
                for kc in range(NKC):
                    for oi in range(4):
                        prep_chunk(band * 4 + oi, kc)

            def make_xT(mt):
                xc = xpool.tile([P, IN_F], f16, tag="xc")
                nc.gpsimd.dma_start(xc[:], x[mt * P : (mt + 1) * P, :])
                xT = xtpool.tile([P, KT, P], f16, tag="xT")
                nc.scalar.dma_start_transpose(xT[:], xc[:])
                return xT

            def mm_band(mt, band, xT):
                ps = mmps.tile([P, 512], f32, tag="ps")
                n0 = band * 512
                for k in range(KT):
                    nc.tensor.matmul(
                        ps[:],
                        xT[:, k, :],
                        B[:, k, n0 : n0 + 512],
                        start=(k == 0),
                        stop=(k == KT - 1),
                    )
                yq = ypool.tile([P, 512], f32, tag="yq")
                nc.vector.tensor_copy(yq[:], ps[:])
                nc.sync.dma_start(
                    y[mt * P : (mt + 1) * P, n0 : n0 + 512], yq[:]
                )

            # Warm phase: band-major so W band b+1 streams in (DMA/DVE only)
            # while the PE runs band b's matmuls.
            prep_band(0)
            warm_xT = [make_xT(mt) for mt in range(WARM)]
            for band in range(NBAND):
                if band + 1 < NBAND:
                    prep_band(band + 1)
                for mt in range(WARM):
                    mm_band(mt, band, warm_xT[mt])

            # steady phase
            for mt in range(WARM, MT):
                xT = make_xT(mt)
                for band in range(NBAND):
                    mm_band(mt, band, xT)

    nc.finalize()
    return nc


def _get_nc():
    if "nc" not in _CACHE:
        _CACHE["nc"] = build()
    return _CACHE["nc"]


def kernel(x, weight, scale):
    global last_result
    nc = _get_nc()
    x = np.ascontiguousarray(np.asarray(x, dtype=np.float32))
    weight = np.ascontiguousarray(np.asarray(weight, dtype=np.float32))
    scale = np.ascontiguousarray(np.asarray(scale, dtype=np.float32))
    in_maps = [
        {
            "x": x,
            "weight": np.ascontiguousarray(weight[c * O_SH : (c + 1) * O_SH]),
            "scale": np.ascontiguousarray(scale[c * O_SH : (c + 1) * O_SH]),
        }
        for c in range(NCORES)
    ]
    res = run_bass_kernel_spmd(nc, in_maps, list(range(NCORES)))
    last_result = res
    return np.concatenate([res.results[c]["y"] for c in range(NCORES)], axis=1)


if __name__ == "__main__":
    rng = np.random.default_rng(0)
    xv = rng.standard_normal((TOKENS, IN_F), dtype=np.float32)
    wv = rng.standard_normal((OUT_F, IN_F), dtype=np.float32)
    sv = np.ones(OUT_F, dtype=np.float32)
    yv = kernel(xv, wv, sv)
    print("out shape:", yv.shape, yv.dtype)
